# revision 31
# baseline (speedup 1.0000x reference)
"""Trainium2 Bass kernel: isometry-regularization loss (nn_IsometryReg).

Math: for a linear classifier l = xW + b (c=10 classes, n=3072 features),
the per-sample Jacobian of y = 2 r[:9] / (1 - r[9])  (r = sqrt(a*softmax(l)+eps))
w.r.t. x factors as jac = Jl @ W^T with Jl [9,10] the Jacobian w.r.t. logits.
Jl has rank-1 + diagonal structure:
    Jl = [diag(alpha) | 0] + gamma e9^T - tau s^T,   tau = alpha + gamma
so with K = W^T W, sK = K s, c3 = s^T K s:
    TT   = Jl K   = alpha_i (K[i,:] - sK) + gamma_i (K[9,:] - sK)
    TTs  = TT s   = alpha*(sK[:9]-c3) + gamma*(sK[9]-c3)
    G    = TT Jl^T:  G[i,l] = alpha_l TT[i,l] + gamma_l TT[i,9] - tau_l TTs[i]
||G - f I||_F^2 = ||G||^2 - 2 f tr(G) + 9 f^2, and
arccos(x) = arctan(sqrt(1/x^2 - 1)) for the x in (0,1] range here.

Device computes RES = ||G - f I||_F^2 per sample; host takes sqrt(RES)/n
and the final mean (the all-reduce step of the data-parallel sharding).

Sharding: pure data-parallel, 128 samples per core on 8 cores; W, b, K
replicated.  Input is a single interleaved bf16 tensor per core:
24 j-blocks of [x_j^T [128x128] | W_j [128x10]] so each DMA chunk carries
matching matmul operands, plus a tail (classifier bias b in bf16, K in raw
f32 bytes read back via bitcast).  Logits land directly in [sample, class]
layout (out = xt_j^T @ W_j), so no transpose / bias-add is needed on the
critical path; the bias is a 1-partition matmul folded into the PSUM
accumulation group.
"""

import numpy as np
import ml_dtypes

import concourse.bass as bass
import concourse.tile as tile
from concourse import mybir
from concourse.bass_utils import run_bass_kernel_spmd

F32 = mybir.dt.float32
FP8 = mybir.dt.float8e4
FP8_NP = ml_dtypes.float8_e4m3
AX = mybir.AxisListType
OP = mybir.AluOpType
AF = mybir.ActivationFunctionType

B, N, C = 1024, 3072, 10
M = C - 1                      # 9
NCORES = 8
BC = B // NCORES               # 128 samples per core
KCH = N // 128                 # 24 k-chunks (j-blocks)
JW = 128 + 2 * C               # 148 cols per j-block (x | W-hi | W-lo residual)
XWCOLS = KCH * JW              # 3552
BCOL = XWCOLS                  # b*WSCALE at [3552, 3562) fp8
KCOL = XWCOLS + C + 2          # K f32 bytes at [3564, 3964), 4B-aligned
NCOLS = KCOL + 4 * C * C       # 3964
NUM_STAB = 1e-4
A_CONST = 1.0 - C * NUM_STAB   # 0.999
EPSILON = 0.1
WSCALE = 64.0                  # host premultiplies W,b so fp8 W avoids subnormals

# dispatch-ordered column ranges; first chunk carries j-blocks 12..23 + b + K
CHUNKS = [(12 * JW, NCOLS), (0, 12 * JW)]
JORDER = list(range(12, 24)) + list(range(0, 12))

_CACHE = {}

USE_POOL = True


def _build():
    nc = bass.Bass()

    xw = nc.dram_tensor("xw", [BC, NCOLS], FP8, kind="ExternalInput")
    # scatter-add dram strides must be 256B multiples -> pad rows to 64 f32
    out = nc.dram_tensor("res", [BC, 64], F32, kind="ExternalOutput")

    with tile.TileContext(nc) as tc:
        with (
            tc.tile_pool(name="const", bufs=1) as const,
            tc.tile_pool(name="xb", bufs=1) as xb,
            tc.tile_pool(name="work", bufs=1) as work,
            tc.tile_pool(name="psum", bufs=1, space="PSUM") as psum,
        ):
            # ---- loads ----
            xw_sb = xb.tile([BC, NCOLS], FP8)
            for lo, hi in CHUNKS:
                nc.sync.dma_start(xw_sb[:, lo:hi], xw[:, lo:hi])

            ones_bf = const.tile([1, BC], FP8)
            nc.gpsimd.memset(ones_bf[:], 1.0)
            ones_f = const.tile([1, BC], F32)
            nc.gpsimd.memset(ones_f[:], 1.0)
            eps_sb = const.tile([BC, 1], F32)
            nc.gpsimd.memset(eps_sb[:], NUM_STAB)

            # ---- output via pre-generated SWDGE descriptors ----
            # The scatter writes RES[i] -> res[idx[i]]; descriptors are built
            # during the load phase, so at RES-ready only a cheap trigger and
            # the tiny transfer remain (vs. full HWDGE dispatch latency).
            # idxs[p, s] = s*16 + p on the first 16 partitions is the identity
            # permutation; other partitions only need values in [-1, 128).
            RES = work.tile([BC, 1], F32)
            idxs_raw = const.tile([BC, BC // 16], mybir.dt.int16)
            nc.gpsimd.iota(idxs_raw[:], pattern=[[16, BC // 16]], base=0,
                           channel_multiplier=1)
            idxs = const.tile([BC, BC // 16], mybir.dt.int16)
            nc.gpsimd.tensor_scalar_min(idxs[:], idxs_raw[:], 127)
            out_sem = nc.alloc_semaphore("res_dma")
            nc.gpsimd.dma_scatter_add(
                out[:, 0:1], RES[:], idxs[:], num_idxs=BC, num_idxs_reg=BC,
                elem_size=1, elem_step=64, prepare_only=True, sem=out_sem,
            )

            # ---- kbc[p, a*10+b] = K[a,b] broadcast to all partitions ----
            kview = xw_sb[0:1, KCOL:NCOLS].bitcast(F32)     # [1, 100]
            kbc_ps = psum.tile([BC, C * C], F32)
            nc.tensor.matmul(kbc_ps[:], ones_f[:], kview, start=True, stop=True)
            kbc = const.tile([BC, C * C], F32)
            nc.scalar.copy(kbc[:], kbc_ps[:])

            # ---- logits [128 samples, 10] = x W + b, accumulated in PSUM ----
            lpsum = psum.tile([BC, C], F32)
            bview = xw_sb[0:1, BCOL:BCOL + C]               # [1, 10] fp8
            nc.tensor.matmul(lpsum[:], ones_bf[:], bview, start=True, stop=False)
            for idx, j in enumerate(JORDER):
                xblk = xw_sb[:, j * JW:j * JW + 128]
                last = idx == KCH - 1
                # W is fp8 hi + fp8 residual, summed in the same PSUM group
                nc.tensor.matmul(
                    lpsum[:], xblk, xw_sb[:, j * JW + 128:j * JW + 128 + C],
                    start=False, stop=False,
                )
                nc.tensor.matmul(
                    lpsum[:], xblk, xw_sb[:, j * JW + 128 + C:(j + 1) * JW],
                    start=False, stop=last,
                )

            # ---- softmax (no max-subtraction: |logits| <~ 6) ----
            # lpsum holds WSCALE * logits; the activation scale undoes it
            E = work.tile([BC, C], F32)
            SE = work.tile([BC, 1], F32)
            nc.scalar.activation(
                E[:], lpsum[:], AF.Exp, scale=1.0 / WSCALE, accum_out=SE[:]
            )
            SEr = work.tile([BC, 1], F32)
            nc.vector.reciprocal(SEr[:], SE[:])
            S = work.tile([BC, C], F32)
            nc.vector.tensor_scalar_mul(S[:], E[:], SEr[:])

            # ---- sK = K s, c3 = s^T K s, and derived consts ----
            SKm = work.tile([BC, C * C], F32)
            nc.vector.tensor_mul(
                SKm[:].rearrange("p (k j) -> p k j", k=C),
                S[:, None, :].broadcast_to([BC, C, C]),
                kbc[:].rearrange("p (k j) -> p k j", k=C),
            )
            sK = work.tile([BC, C], F32)
            nc.vector.tensor_reduce(
                sK[:], SKm[:].rearrange("p (k j) -> p k j", k=C),
                axis=AX.X, op=OP.add,
            )
            c3s = work.tile([BC, C], F32)
            c3 = work.tile([BC, 1], F32)
            nc.vector.scalar_tensor_tensor(
                c3s[:], sK[:], 1.0, S[:], op0=OP.mult, op1=OP.mult,
                accum_out=c3[:],
            )
            veng = nc.gpsimd if USE_POOL else nc.vector
            E1 = work.tile([BC, M], F32)
            veng.tensor_scalar_sub(E1[:], sK[:, 0:M], c3[:])
            e2 = work.tile([BC, 1], F32)
            veng.tensor_scalar_sub(e2[:], sK[:, M:C], c3[:])
            D1 = work.tile([BC, M * C], F32)
            veng.tensor_sub(
                D1[:].rearrange("p (i k) -> p i k", i=M),
                kbc[:, 0:M * C].rearrange("p (i k) -> p i k", i=M),
                sK[:, None, :].broadcast_to([BC, M, C]),
            )
            D2 = work.tile([BC, C], F32)
            veng.tensor_sub(D2[:], kbc[:, M * C:C * C], sK[:])

            # ---- r = sqrt(a*s + eps), SR = sum r ----
            R = work.tile([BC, C], F32)
            SR = work.tile([BC, 1], F32)
            nc.scalar.activation(
                R[:], S[:], AF.Sqrt, bias=eps_sb[:], scale=A_CONST,
                accum_out=SR[:],
            )
            Rinv = work.tile([BC, C], F32)
            nc.vector.reciprocal(Rinv[:], R[:])

            # delta branch: arccos(SR/sqrt(10)) = arctan(sqrt(10/SR^2 - 1))
            SRinv = work.tile([BC, 1], F32)
            nc.vector.reciprocal(SRinv[:], SR[:])
            QQ = work.tile([BC, 1], F32)
            nc.vector.tensor_mul(QQ[:], SRinv[:], SRinv[:])
            ARGt = work.tile([BC, 1], F32)
            nc.vector.tensor_scalar(
                ARGt[:], QQ[:], float(C), -1.0, op0=OP.mult, op1=OP.add
            )
            ARGin = work.tile([BC, 1], F32)
            nc.vector.tensor_scalar_max(ARGin[:], ARGt[:], 0.0)
            ARG = work.tile([BC, 1], F32)
            nc.scalar.activation(ARG[:], ARGin[:], AF.Sqrt)
            AC = work.tile([BC, 1], F32)
            nc.scalar.activation(AC[:], ARG[:], AF.Arctan)
            FA = work.tile([BC, 1], F32)
            nc.scalar.activation(FA[:], AC[:], AF.Square)

            # u = 1/(1 - r9), alpha, gamma, tau
            OMR = work.tile([BC, 1], F32)
            nc.vector.tensor_scalar(
                OMR[:], R[:, M:C], -1.0, 1.0, op0=OP.mult, op1=OP.add
            )
            U = work.tile([BC, 1], F32)
            nc.vector.reciprocal(U[:], OMR[:])
            U2 = work.tile([BC, 1], F32)
            nc.vector.tensor_mul(U2[:], U[:], U[:])
            SRi = work.tile([BC, M], F32)
            nc.vector.tensor_mul(SRi[:], S[:, 0:M], Rinv[:, 0:M])
            ALPHA = work.tile([BC, M], F32)
            nc.vector.tensor_scalar(
                ALPHA[:], SRi[:], U[:], A_CONST, op0=OP.mult, op1=OP.mult
            )
            SR9 = work.tile([BC, 1], F32)
            nc.vector.tensor_mul(SR9[:], S[:, M:C], Rinv[:, M:C])
            G0 = work.tile([BC, 1], F32)
            nc.vector.tensor_scalar(
                G0[:], SR9[:], U2[:], A_CONST, op0=OP.mult, op1=OP.mult
            )
            GAMMA = work.tile([BC, M], F32)
            nc.vector.tensor_scalar_mul(GAMMA[:], R[:, 0:M], G0[:])
            TAU = work.tile([BC, M], F32)
            nc.vector.tensor_add(TAU[:], ALPHA[:], GAMMA[:])

            # ---- TT = Jl K  [128, 90] ----
            M1 = work.tile([BC, M * C], F32)
            nc.vector.tensor_mul(
                M1[:].rearrange("p (i k) -> p i k", i=M),
                ALPHA[:, :, None].broadcast_to([BC, M, C]),
                D1[:].rearrange("p (i k) -> p i k", i=M),
            )
            TT = work.tile([BC, M * C], F32)
            nc.vector.scalar_tensor_tensor(
                TT[:].rearrange("p (i k) -> p i k", i=M),
                GAMMA[:, :, None].broadcast_to([BC, M, C]),
                1.0,
                D2[:, None, :].broadcast_to([BC, M, C]),
                op0=OP.mult, op1=OP.mult,
            )
            nc.vector.tensor_add(TT[:], TT[:], M1[:])

            # TTs = TT s  [128, 9]
            t1 = work.tile([BC, M], F32)
            nc.vector.tensor_mul(t1[:], ALPHA[:], E1[:])
            t2 = work.tile([BC, M], F32)
            nc.vector.tensor_scalar_mul(t2[:], GAMMA[:], e2[:])
            TTs = work.tile([BC, M], F32)
            nc.vector.tensor_add(TTs[:], t1[:], t2[:])

            # ---- G = TT Jl^T  [128, 81] ----
            g3 = work.tile([BC, M * M], F32)
            nc.vector.tensor_mul(
                g3[:].rearrange("p (i l) -> p i l", i=M),
                TTs[:, :, None].broadcast_to([BC, M, M]),
                TAU[:, None, :].broadcast_to([BC, M, M]),
            )
            g1 = work.tile([BC, M * M], F32)
            nc.vector.tensor_mul(
                g1[:].rearrange("p (i l) -> p i l", i=M),
                TT[:].rearrange("p (i k) -> p i k", i=M)[:, :, 0:M],
                ALPHA[:, None, :].broadcast_to([BC, M, M]),
            )
            g2 = work.tile([BC, M * M], F32)
            nc.vector.tensor_mul(
                g2[:].rearrange("p (i l) -> p i l", i=M),
                TT[:, M:M * C:C][:, :, None].broadcast_to([BC, M, M]),
                GAMMA[:, None, :].broadcast_to([BC, M, M]),
            )
            g12 = work.tile([BC, M * M], F32)
            nc.vector.tensor_add(g12[:], g1[:], g2[:])
            G = work.tile([BC, M * M], F32)
            nc.vector.tensor_sub(G[:], g12[:], g3[:])

            # ---- ||G||^2, tr(G) ----
            GG = work.tile([BC, M * M], F32)
            SSQ = work.tile([BC, 1], F32)
            nc.vector.scalar_tensor_tensor(
                GG[:], G[:], 1.0, G[:], op0=OP.mult, op1=OP.mult,
                accum_out=SSQ[:],
            )
            TRG = work.tile([BC, 1], F32)
            nc.vector.tensor_reduce(
                TRG[:], G[:, 0:M * M:M + 1], axis=AX.X, op=OP.add
            )

            # ---- f = 100 * AC^2 * u^2 ; RES = SSQ - 2 f trG + 9 f^2 ----
            F = work.tile([BC, 1], F32)
            nc.vector.tensor_scalar(
                F[:], FA[:], U2[:], 100.0, op0=OP.mult, op1=OP.mult
            )
            FF = work.tile([BC, 1], F32)
            nc.vector.tensor_mul(FF[:], F[:], F[:])
            FT = work.tile([BC, 1], F32)
            nc.vector.tensor_mul(FT[:], F[:], TRG[:])
            R1 = work.tile([BC, 1], F32)
            nc.vector.scalar_tensor_tensor(
                R1[:], FT[:], -2.0, SSQ[:], op0=OP.mult, op1=OP.add
            )
            # RES on Pool so the trigger (also Pool) follows it in program
            # order with no extra cross-engine hand-off
            nc.gpsimd.scalar_tensor_tensor(
                RES[:], FF[:], 9.0, R1[:], op0=OP.mult, op1=OP.add
            )
            nc.gpsimd.trigger_dma(count=None)
            wait_ins = nc.gpsimd.wait_ge(out_sem, 1)
            _CACHE["res_wait_name"] = wait_ins.ins.name

    return nc


def _fix_swdge_accounting(nc):
    """Tile's global clock ticks the DMASW lane for the prepare-only scatter,
    expecting the DMA descriptor to increment the DMASW queue sem at
    completion; with a user completion sem (`sem=`), that increment never
    fires and the final drain deadlocks.  Attach the expected increment to
    our explicit completion-wait instruction instead — it executes only
    after the data has landed, so the ordering the drain relies on holds.
    """
    waits_needed = {}
    updated = set()
    for blk in nc.main_func.blocks:
        for ins in blk.instructions:
            si = getattr(ins, "sync_info", None)
            if si is None:
                continue
            for w in si.on_wait or []:
                if str(getattr(w, "ant_name", "") or "").startswith("DMASW"):
                    waits_needed[w.id] = (w, w.wait_value or 0)
            for u in si.on_update or []:
                updated.add(u.id)
    missing = [(w, v) for sid, (w, v) in waits_needed.items() if sid not in updated]
    if not missing:
        return nc
    wait_name = _CACHE.get("res_wait_name")
    for blk in nc.main_func.blocks:
        for ins in blk.instructions:
            if ins.name == wait_name:
                si = ins.sync_info or mybir.SyncInfo(on_wait=[], on_update=[])
                for w, v in missing:
                    si.on_update = list(si.on_update or []) + [
                        mybir.SyncUpdate(
                            sync_type=w.sync_type,
                            id=w.id,
                            ant_name=w.ant_name,
                            update_mode="sem-add-imm",
                            update_value=v,
                        )
                    ]
                ins.sync_info = si
                return nc
    raise AssertionError("res_dma completion-wait instruction not found")


def _split_waits(nc):
    """Walrus codegen on this toolchain encodes at most one sync-wait per
    instruction; hoist extra waits onto same-engine NoOps inserted before."""
    for blk in nc.main_func.blocks:
        newlist = []
        changed = False
        for ins in blk.instructions:
            si = getattr(ins, "sync_info", None)
            ow = getattr(si, "on_wait", None) if si is not None else None
            if ow and len(ow) > 1:
                for idx, w in enumerate(ow[:-1]):
                    nop = mybir.InstNoOp(name=f"{ins.name}-sw{idx}", ins=[], outs=[])
                    nop.engine = ins.engine
                    nop.sync_info = mybir.SyncInfo(on_wait=[w], on_update=[])
                    newlist.append(nop)
                si.on_wait = [ow[-1]]
                changed = True
            newlist.append(ins)
        if changed:
            blk.instructions = newlist
    return nc


def _get_nc():
    if "nc" not in _CACHE:
        _CACHE["nc"] = _split_waits(_fix_swdge_accounting(_build()))
    return _CACHE["nc"]


def _shard_inputs(data, W, b):
    """Host-side layout: interleaved transposed-x / W chunks + packed consts."""
    x = np.ascontiguousarray(np.asarray(data, np.float32).reshape(B, N))
    W = np.asarray(W, np.float32)
    b = np.asarray(b, np.float32)
    K = np.ascontiguousarray(W.T @ W)                        # [10, 10] f32

    shared = np.zeros((BC, NCOLS), dtype=FP8_NP)
    Whi = (W * WSCALE).astype(FP8_NP)
    Wlo = (W * WSCALE - Whi.astype(np.float32)).astype(FP8_NP)
    for j in range(KCH):
        shared[:, j * JW + 128:j * JW + 128 + C] = Whi[j * 128:(j + 1) * 128]
        shared[:, j * JW + 128 + C:(j + 1) * JW] = Wlo[j * 128:(j + 1) * 128]
    shared[0, BCOL:BCOL + C] = (b * WSCALE).astype(FP8_NP)
    shared.view(np.uint8)[0, KCOL:NCOLS] = K.ravel().view(np.uint8)

    in_maps = []
    for i in range(NCORES):
        sh = x[i * BC:(i + 1) * BC]                          # [128, 3072]
        # xt[p, (j, b)] = sh[b, j*128 + p]
        xt = sh.reshape(BC, KCH, 128).transpose(2, 1, 0)     # [128, 24, 128]
        xw = shared.copy()
        for j in range(KCH):
            xw[:, j * JW:j * JW + 128] = xt[:, j, :].astype(FP8_NP)
        in_maps.append({"xw": xw})
    return in_maps


def kernel(data, W, b, trace=False, trace_kwargs=None):
    nc = _get_nc()
    in_maps = _shard_inputs(np.asarray(data), np.asarray(W), np.asarray(b))
    kw = {}
    if trace:
        kw = dict(trace=True, trace_cores=list(range(NCORES)),
                  stitch_traces=True)
        if trace_kwargs:
            kw["trace_kwargs"] = trace_kwargs
    res = run_bass_kernel_spmd(
        nc, in_maps, core_ids=list(range(NCORES)), **kw
    )
    ress = np.concatenate([r["res"][:, 0].reshape(-1) for r in res.results])
    regs = np.sqrt(np.maximum(ress.astype(np.float64), 0.0)) / float(N)
    mean = np.float32(regs.mean())
    out = (np.asarray(mean, np.float32), np.asarray(0, np.int32))
    if trace:
        return out, res
    return out


# revision 36
# speedup vs baseline: 1.1821x; 1.1821x over previous
"""Trainium2 Bass kernel: isometry-regularization loss (nn_IsometryReg).

Math: for a linear classifier l = xW + b (c=10 classes, n=3072 features),
the per-sample Jacobian of y = 2 r[:9] / (1 - r[9])  (r = sqrt(a*softmax(l)+eps))
w.r.t. x factors as jac = Jl @ W^T with Jl [9,10] the Jacobian w.r.t. logits.
Jl has rank-1 + diagonal structure:
    Jl = [diag(alpha) | 0] + gamma e9^T - tau s^T,   tau = alpha + gamma
so with K = W^T W, sK = K s, c3 = s^T K s:
    TT   = Jl K   = alpha_i (K[i,:] - sK) + gamma_i (K[9,:] - sK)
    TTs  = TT s   = alpha*(sK[:9]-c3) + gamma*(sK[9]-c3)
    G    = TT Jl^T:  G[i,l] = alpha_l TT[i,l] + gamma_l TT[i,9] - tau_l TTs[i]
||G - f I||_F^2 = ||G||^2 - 2 f tr(G) + 9 f^2, and
arccos(x) = arctan(sqrt(1/x^2 - 1)) for the x in (0,1] range here.

Device computes RES = ||G - f I||_F^2 per sample; host takes sqrt(RES)/n
and the final mean (the all-reduce step of the data-parallel sharding).

Sharding: pure data-parallel, 128 samples per core on 8 cores; W, b, K
replicated.  Input is a single interleaved bf16 tensor per core:
24 j-blocks of [x_j^T [128x128] | W_j [128x10]] so each DMA chunk carries
matching matmul operands, plus a tail (classifier bias b in bf16, K in raw
f32 bytes read back via bitcast).  Logits land directly in [sample, class]
layout (out = xt_j^T @ W_j), so no transpose / bias-add is needed on the
critical path; the bias is a 1-partition matmul folded into the PSUM
accumulation group.
"""

import numpy as np
import ml_dtypes

import concourse.bass as bass
import concourse.tile as tile
from concourse import mybir
from concourse.bass_utils import run_bass_kernel_spmd

F32 = mybir.dt.float32
FP8 = mybir.dt.float8e4
FP8_NP = ml_dtypes.float8_e4m3
AX = mybir.AxisListType
OP = mybir.AluOpType
AF = mybir.ActivationFunctionType

B, N, C = 1024, 3072, 10
M = C - 1                      # 9
NCORES = 8
BC = B // NCORES               # 128 samples per core
KCH = N // 128                 # 24 k-chunks (j-blocks)
JW = 128 + 2 * C               # 148 cols per j-block (x | W-hi | W-lo residual)
XWCOLS = KCH * JW              # 3552
BCOL = XWCOLS                  # b*WSCALE at [3552, 3562) fp8
KCOL = XWCOLS + C + 2          # K f32 bytes at [3564, 3964), 4B-aligned
NCOLS = KCOL + 4 * C * C       # 3964
NUM_STAB = 1e-4
A_CONST = 1.0 - C * NUM_STAB   # 0.999
EPSILON = 0.1
WSCALE = 64.0                  # host premultiplies W,b so fp8 W avoids subnormals

# dispatch-ordered column ranges; first chunk carries j-blocks 12..23 + b + K
CHUNKS = [(12 * JW, NCOLS), (0, 12 * JW)]
JORDER = list(range(12, 24)) + list(range(0, 12))

_CACHE = {}

USE_POOL = True


def _build():
    nc = bass.Bass()

    xw = nc.dram_tensor("xw", [BC, NCOLS], FP8, kind="ExternalInput")
    # scatter-add dram strides must be 256B multiples -> pad rows to 64 f32
    out = nc.dram_tensor("res", [BC, 64], F32, kind="ExternalOutput")

    with tile.TileContext(nc) as tc:
        with (
            tc.tile_pool(name="const", bufs=1) as const,
            tc.tile_pool(name="xb", bufs=1) as xb,
            tc.tile_pool(name="work", bufs=1) as work,
            tc.tile_pool(name="psum", bufs=1, space="PSUM") as psum,
        ):
            # ---- loads ----
            xw_sb = xb.tile([BC, NCOLS], FP8)
            for lo, hi in CHUNKS:
                nc.sync.dma_start(xw_sb[:, lo:hi], xw[:, lo:hi])

            ones_bf = const.tile([1, BC], FP8)
            nc.gpsimd.memset(ones_bf[:], 1.0)
            ones_f = const.tile([1, BC], F32)
            nc.gpsimd.memset(ones_f[:], 1.0)
            eps_sb = const.tile([BC, 1], F32)
            nc.gpsimd.memset(eps_sb[:], NUM_STAB)

            # ---- output via pre-generated SWDGE descriptors ----
            # The scatter writes RES[i] -> res[idx[i]]; descriptors are built
            # during the load phase, so at RES-ready only a cheap trigger and
            # the tiny transfer remain (vs. full HWDGE dispatch latency).
            # idxs[p, s] = s*16 + p on the first 16 partitions is the identity
            # permutation; other partitions only need values in [-1, 128).
            RES = work.tile([BC, 1], F32)
            decoy = work.tile([BC, 1], F32)
            nc.gpsimd.memset(decoy[:], 0.0)
            idxs_raw = const.tile([BC, BC // 16], mybir.dt.int16)
            nc.gpsimd.iota(idxs_raw[:], pattern=[[16, BC // 16]], base=0,
                           channel_multiplier=1)
            idxs = const.tile([BC, BC // 16], mybir.dt.int16)
            nc.gpsimd.tensor_scalar_min(idxs[:], idxs_raw[:], 127)
            out_sem = nc.alloc_semaphore("res_dma")
            # The prep nominally reads `decoy` so the scheduler runs its
            # ~1us descriptor generation during the load phase; the source
            # AP is retargeted to RES post-lowering (_retarget_scatter).
            # Ordering of the actual data read is the trigger, which sits
            # after the RES producer in Pool program order.
            prep_ins = nc.gpsimd.dma_scatter_add(
                out[:, 0:1], decoy[:], idxs[:], num_idxs=BC, num_idxs_reg=BC,
                elem_size=1, elem_step=64, prepare_only=True, sem=out_sem,
            )
            _CACHE["prep_name"] = prep_ins.ins.name

            # ---- kbc[p, a*10+b] = K[a,b] broadcast to all partitions ----
            kview = xw_sb[0:1, KCOL:NCOLS].bitcast(F32)     # [1, 100]
            kbc_ps = psum.tile([BC, C * C], F32)
            nc.tensor.matmul(kbc_ps[:], ones_f[:], kview, start=True, stop=True)
            kbc = const.tile([BC, C * C], F32)
            nc.scalar.copy(kbc[:], kbc_ps[:])

            # ---- logits [128 samples, 10] = x W + b, accumulated in PSUM ----
            lpsum = psum.tile([BC, C], F32)
            bview = xw_sb[0:1, BCOL:BCOL + C]               # [1, 10] fp8
            nc.tensor.matmul(lpsum[:], ones_bf[:], bview, start=True, stop=False)
            for idx, j in enumerate(JORDER):
                xblk = xw_sb[:, j * JW:j * JW + 128]
                last = idx == KCH - 1
                # W is fp8 hi + fp8 residual, summed in the same PSUM group
                nc.tensor.matmul(
                    lpsum[:], xblk, xw_sb[:, j * JW + 128:j * JW + 128 + C],
                    start=False, stop=False,
                )
                nc.tensor.matmul(
                    lpsum[:], xblk, xw_sb[:, j * JW + 128 + C:(j + 1) * JW],
                    start=False, stop=last,
                )

            # ---- softmax (no max-subtraction: |logits| <~ 6) ----
            # lpsum holds WSCALE * logits; the activation scale undoes it
            E = work.tile([BC, C], F32)
            SE = work.tile([BC, 1], F32)
            nc.scalar.activation(
                E[:], lpsum[:], AF.Exp, scale=1.0 / WSCALE, accum_out=SE[:]
            )
            SEr = work.tile([BC, 1], F32)
            nc.vector.reciprocal(SEr[:], SE[:])
            S = work.tile([BC, C], F32)
            nc.vector.tensor_scalar_mul(S[:], E[:], SEr[:])

            # ---- sK = K s, c3 = s^T K s, and derived consts ----
            SKm = work.tile([BC, C * C], F32)
            nc.vector.tensor_mul(
                SKm[:].rearrange("p (k j) -> p k j", k=C),
                S[:, None, :].broadcast_to([BC, C, C]),
                kbc[:].rearrange("p (k j) -> p k j", k=C),
            )
            sK = work.tile([BC, C], F32)
            nc.vector.tensor_reduce(
                sK[:], SKm[:].rearrange("p (k j) -> p k j", k=C),
                axis=AX.X, op=OP.add,
            )
            c3s = work.tile([BC, C], F32)
            c3 = work.tile([BC, 1], F32)
            nc.vector.scalar_tensor_tensor(
                c3s[:], sK[:], 1.0, S[:], op0=OP.mult, op1=OP.mult,
                accum_out=c3[:],
            )
            veng = nc.gpsimd if USE_POOL else nc.vector
            E1 = work.tile([BC, M], F32)
            veng.tensor_scalar_sub(E1[:], sK[:, 0:M], c3[:])
            e2 = work.tile([BC, 1], F32)
            veng.tensor_scalar_sub(e2[:], sK[:, M:C], c3[:])
            D1 = work.tile([BC, M * C], F32)
            veng.tensor_sub(
                D1[:].rearrange("p (i k) -> p i k", i=M),
                kbc[:, 0:M * C].rearrange("p (i k) -> p i k", i=M),
                sK[:, None, :].broadcast_to([BC, M, C]),
            )
            D2 = work.tile([BC, C], F32)
            veng.tensor_sub(D2[:], kbc[:, M * C:C * C], sK[:])

            # ---- r = sqrt(a*s + eps), SR = sum r ----
            R = work.tile([BC, C], F32)
            SR = work.tile([BC, 1], F32)
            nc.scalar.activation(
                R[:], S[:], AF.Sqrt, bias=eps_sb[:], scale=A_CONST,
                accum_out=SR[:],
            )
            Rinv = work.tile([BC, C], F32)
            nc.vector.reciprocal(Rinv[:], R[:])

            # delta branch: arccos(SR/sqrt(10)) = arctan(sqrt(10/SR^2 - 1))
            SRinv = work.tile([BC, 1], F32)
            nc.vector.reciprocal(SRinv[:], SR[:])
            QQ = work.tile([BC, 1], F32)
            nc.vector.tensor_mul(QQ[:], SRinv[:], SRinv[:])
            ARGt = work.tile([BC, 1], F32)
            nc.vector.tensor_scalar(
                ARGt[:], QQ[:], float(C), -1.0, op0=OP.mult, op1=OP.add
            )
            ARGin = work.tile([BC, 1], F32)
            nc.vector.tensor_scalar_max(ARGin[:], ARGt[:], 0.0)
            ARG = work.tile([BC, 1], F32)
            nc.scalar.activation(ARG[:], ARGin[:], AF.Sqrt)
            AC = work.tile([BC, 1], F32)
            nc.scalar.activation(AC[:], ARG[:], AF.Arctan)
            FA = work.tile([BC, 1], F32)
            nc.scalar.activation(FA[:], AC[:], AF.Square)

            # u = 1/(1 - r9), alpha, gamma, tau
            OMR = work.tile([BC, 1], F32)
            nc.vector.tensor_scalar(
                OMR[:], R[:, M:C], -1.0, 1.0, op0=OP.mult, op1=OP.add
            )
            U = work.tile([BC, 1], F32)
            nc.vector.reciprocal(U[:], OMR[:])
            U2 = work.tile([BC, 1], F32)
            nc.vector.tensor_mul(U2[:], U[:], U[:])
            SRi = work.tile([BC, M], F32)
            nc.vector.tensor_mul(SRi[:], S[:, 0:M], Rinv[:, 0:M])
            ALPHA = work.tile([BC, M], F32)
            nc.vector.tensor_scalar(
                ALPHA[:], SRi[:], U[:], A_CONST, op0=OP.mult, op1=OP.mult
            )
            SR9 = work.tile([BC, 1], F32)
            nc.vector.tensor_mul(SR9[:], S[:, M:C], Rinv[:, M:C])
            G0 = work.tile([BC, 1], F32)
            nc.vector.tensor_scalar(
                G0[:], SR9[:], U2[:], A_CONST, op0=OP.mult, op1=OP.mult
            )
            GAMMA = work.tile([BC, M], F32)
            nc.vector.tensor_scalar_mul(GAMMA[:], R[:, 0:M], G0[:])
            TAU = work.tile([BC, M], F32)
            nc.vector.tensor_add(TAU[:], ALPHA[:], GAMMA[:])

            # ---- TT = Jl K  [128, 90] ----
            M1 = work.tile([BC, M * C], F32)
            nc.vector.tensor_mul(
                M1[:].rearrange("p (i k) -> p i k", i=M),
                ALPHA[:, :, None].broadcast_to([BC, M, C]),
                D1[:].rearrange("p (i k) -> p i k", i=M),
            )
            TT = work.tile([BC, M * C], F32)
            nc.vector.scalar_tensor_tensor(
                TT[:].rearrange("p (i k) -> p i k", i=M),
                GAMMA[:, :, None].broadcast_to([BC, M, C]),
                1.0,
                D2[:, None, :].broadcast_to([BC, M, C]),
                op0=OP.mult, op1=OP.mult,
            )
            nc.vector.tensor_add(TT[:], TT[:], M1[:])

            # TTs = TT s  [128, 9]
            t1 = work.tile([BC, M], F32)
            nc.vector.tensor_mul(t1[:], ALPHA[:], E1[:])
            t2 = work.tile([BC, M], F32)
            nc.vector.tensor_scalar_mul(t2[:], GAMMA[:], e2[:])
            TTs = work.tile([BC, M], F32)
            nc.vector.tensor_add(TTs[:], t1[:], t2[:])

            # ---- G = TT Jl^T  [128, 81] ----
            g3 = work.tile([BC, M * M], F32)
            nc.vector.tensor_mul(
                g3[:].rearrange("p (i l) -> p i l", i=M),
                TTs[:, :, None].broadcast_to([BC, M, M]),
                TAU[:, None, :].broadcast_to([BC, M, M]),
            )
            g1 = work.tile([BC, M * M], F32)
            nc.vector.tensor_mul(
                g1[:].rearrange("p (i l) -> p i l", i=M),
                TT[:].rearrange("p (i k) -> p i k", i=M)[:, :, 0:M],
                ALPHA[:, None, :].broadcast_to([BC, M, M]),
            )
            g2 = work.tile([BC, M * M], F32)
            nc.vector.tensor_mul(
                g2[:].rearrange("p (i l) -> p i l", i=M),
                TT[:, M:M * C:C][:, :, None].broadcast_to([BC, M, M]),
                GAMMA[:, None, :].broadcast_to([BC, M, M]),
            )
            g12 = work.tile([BC, M * M], F32)
            nc.vector.tensor_add(g12[:], g1[:], g2[:])
            G = work.tile([BC, M * M], F32)
            nc.vector.tensor_sub(G[:], g12[:], g3[:])

            # ---- ||G||^2, tr(G) ----
            GG = work.tile([BC, M * M], F32)
            SSQ = work.tile([BC, 1], F32)
            nc.vector.scalar_tensor_tensor(
                GG[:], G[:], 1.0, G[:], op0=OP.mult, op1=OP.mult,
                accum_out=SSQ[:],
            )
            TRG = work.tile([BC, 1], F32)
            nc.vector.tensor_reduce(
                TRG[:], G[:, 0:M * M:M + 1], axis=AX.X, op=OP.add
            )

            # ---- f = 100 * AC^2 * u^2 ; RES = SSQ - 2 f trG + 9 f^2 ----
            F = work.tile([BC, 1], F32)
            nc.vector.tensor_scalar(
                F[:], FA[:], U2[:], 100.0, op0=OP.mult, op1=OP.mult
            )
            FF = work.tile([BC, 1], F32)
            nc.vector.tensor_mul(FF[:], F[:], F[:])
            FT = work.tile([BC, 1], F32)
            nc.vector.tensor_mul(FT[:], F[:], TRG[:])
            R1 = work.tile([BC, 1], F32)
            nc.vector.scalar_tensor_tensor(
                R1[:], FT[:], -2.0, SSQ[:], op0=OP.mult, op1=OP.add
            )
            # RES on Pool so the trigger (also Pool) follows it in program
            # order with no extra cross-engine hand-off
            res_ins = nc.gpsimd.scalar_tensor_tensor(
                RES[:], FF[:], 9.0, R1[:], op0=OP.mult, op1=OP.add
            )
            _CACHE["res_prod_name"] = res_ins.ins.name
            nc.gpsimd.trigger_dma(count=None)
            wait_ins = nc.gpsimd.wait_ge(out_sem, 1)
            _CACHE["res_wait_name"] = wait_ins.ins.name

    return nc


def _retarget_scatter(nc):
    """Point the scatter prep's source AP at RES (see _build comment)."""
    prep = res_prod = None
    for blk in nc.main_func.blocks:
        for ins in blk.instructions:
            if ins.name == _CACHE.get("prep_name"):
                prep = ins
            elif ins.name == _CACHE.get("res_prod_name"):
                res_prod = ins
    assert prep is not None and res_prod is not None
    prep.ins[0] = res_prod.outs[0]
    return nc


def _fix_swdge_accounting(nc):
    """Tile's global clock ticks the DMASW lane for the prepare-only scatter,
    expecting the DMA descriptor to increment the DMASW queue sem at
    completion; with a user completion sem (`sem=`), that increment never
    fires and the final drain deadlocks.  Attach the expected increment to
    our explicit completion-wait instruction instead — it executes only
    after the data has landed, so the ordering the drain relies on holds.
    """
    waits_needed = {}
    updated = set()
    for blk in nc.main_func.blocks:
        for ins in blk.instructions:
            si = getattr(ins, "sync_info", None)
            if si is None:
                continue
            for w in si.on_wait or []:
                if str(getattr(w, "ant_name", "") or "").startswith("DMASW"):
                    waits_needed[w.id] = (w, w.wait_value or 0)
            for u in si.on_update or []:
                updated.add(u.id)
    missing = [(w, v) for sid, (w, v) in waits_needed.items() if sid not in updated]
    if not missing:
        return nc
    wait_name = _CACHE.get("res_wait_name")
    for blk in nc.main_func.blocks:
        for ins in blk.instructions:
            if ins.name == wait_name:
                si = ins.sync_info or mybir.SyncInfo(on_wait=[], on_update=[])
                for w, v in missing:
                    si.on_update = list(si.on_update or []) + [
                        mybir.SyncUpdate(
                            sync_type=w.sync_type,
                            id=w.id,
                            ant_name=w.ant_name,
                            update_mode="sem-add-imm",
                            update_value=v,
                        )
                    ]
                ins.sync_info = si
                return nc
    raise AssertionError("res_dma completion-wait instruction not found")


def _split_waits(nc):
    """Walrus codegen on this toolchain encodes at most one sync-wait per
    instruction; hoist extra waits onto same-engine NoOps inserted before."""
    for blk in nc.main_func.blocks:
        newlist = []
        changed = False
        for ins in blk.instructions:
            si = getattr(ins, "sync_info", None)
            ow = getattr(si, "on_wait", None) if si is not None else None
            if ow and len(ow) > 1:
                for idx, w in enumerate(ow[:-1]):
                    nop = mybir.InstNoOp(name=f"{ins.name}-sw{idx}", ins=[], outs=[])
                    nop.engine = ins.engine
                    nop.sync_info = mybir.SyncInfo(on_wait=[w], on_update=[])
                    newlist.append(nop)
                si.on_wait = [ow[-1]]
                changed = True
            newlist.append(ins)
        if changed:
            blk.instructions = newlist
    return nc


def _get_nc():
    if "nc" not in _CACHE:
        _CACHE["nc"] = _split_waits(
            _fix_swdge_accounting(_retarget_scatter(_build()))
        )
    return _CACHE["nc"]


def _shard_inputs(data, W, b):
    """Host-side layout: interleaved transposed-x / W chunks + packed consts."""
    x = np.ascontiguousarray(np.asarray(data, np.float32).reshape(B, N))
    W = np.asarray(W, np.float32)
    b = np.asarray(b, np.float32)
    K = np.ascontiguousarray(W.T @ W)                        # [10, 10] f32

    shared = np.zeros((BC, NCOLS), dtype=FP8_NP)
    Whi = (W * WSCALE).astype(FP8_NP)
    Wlo = (W * WSCALE - Whi.astype(np.float32)).astype(FP8_NP)
    for j in range(KCH):
        shared[:, j * JW + 128:j * JW + 128 + C] = Whi[j * 128:(j + 1) * 128]
        shared[:, j * JW + 128 + C:(j + 1) * JW] = Wlo[j * 128:(j + 1) * 128]
    shared[0, BCOL:BCOL + C] = (b * WSCALE).astype(FP8_NP)
    shared.view(np.uint8)[0, KCOL:NCOLS] = K.ravel().view(np.uint8)

    in_maps = []
    for i in range(NCORES):
        sh = x[i * BC:(i + 1) * BC]                          # [128, 3072]
        # xt[p, (j, b)] = sh[b, j*128 + p]
        xt = sh.reshape(BC, KCH, 128).transpose(2, 1, 0)     # [128, 24, 128]
        xw = shared.copy()
        for j in range(KCH):
            xw[:, j * JW:j * JW + 128] = xt[:, j, :].astype(FP8_NP)
        in_maps.append({"xw": xw})
    return in_maps


def kernel(data, W, b, trace=False, trace_kwargs=None):
    nc = _get_nc()
    in_maps = _shard_inputs(np.asarray(data), np.asarray(W), np.asarray(b))
    kw = {}
    if trace:
        kw = dict(trace=True, trace_cores=list(range(NCORES)),
                  stitch_traces=True)
        if trace_kwargs:
            kw["trace_kwargs"] = trace_kwargs
    res = run_bass_kernel_spmd(
        nc, in_maps, core_ids=list(range(NCORES)), **kw
    )
    ress = np.concatenate([r["res"][:, 0].reshape(-1) for r in res.results])
    regs = np.sqrt(np.maximum(ress.astype(np.float64), 0.0)) / float(N)
    mean = np.float32(regs.mean())
    out = (np.asarray(mean, np.float32), np.asarray(0, np.int32))
    if trace:
        return out, res
    return out
